# revision 27
# baseline (speedup 1.0000x reference)
"""Multi-head attention block on 8 TRN2 NeuronCores.

Sharding: core c -> (batch b = c//2, head-group hg = c%2).
Each core computes QKV projections for its 8 heads over its batch
(fp32r matmuls), attention (fp32r QK^T, exp on ACT, bf16 A@V with a
col-packed ones-matmul producing replicated row-sums), and a bf16
output projection of its head-group's channels. Pairs of cores
(same batch) combine partial projections with per-q-block
ReduceScatter collectives; the host concatenates the 8 per-core
output shards into the full [4, 2048, 1024] result.

Schedule notes: K/V projections form a dense PE preamble; the Q
projection is computed per-q-block inside the attention loop (keeps
the PE near-saturated so the HAM clock gate stays at full rate).
The exp pipeline is staggered head-by-head through a 2-deep PSUM
pool; the previous q-block's output projection and ReduceScatter are
interleaved into the current q-block's head loop through a dedicated
PSUM pool so they never contend with the z accumulators.
"""

import sys

if "/opt/trn_rl_repo" not in sys.path:
    sys.path.insert(0, "/opt/trn_rl_repo")

import numpy as np
import ml_dtypes

N_CORES = 8
B, T, DIM = 4, 2048, 1024
H_TOT, HD = 16, 64
HPC = H_TOT // 2          # heads per core (2 head-groups)
DQ = HPC * HD             # 512: per-core q/k/v width
SCALE = HD ** -0.5
CH = 2                    # k-blocks per exp chunk
KB_T = T // 128           # 16 k-blocks over sequence
KB_C = DIM // 128         # 8 k-blocks over channel dim

_CACHE = {}


def _build():
    import concourse.bass as bass
    import concourse.tile as tile
    from concourse import bacc, mybir

    F32 = mybir.dt.float32
    F32R = mybir.dt.float32r
    BF16 = mybir.dt.bfloat16
    AF = mybir.ActivationFunctionType

    nc = bacc.Bacc("TRN2", target_bir_lowering=False, debug=False,
                   num_devices=N_CORES)

    x_t = nc.dram_tensor("x_t", [DIM, T], BF16, kind="ExternalInput").ap()
    w_qkv = nc.dram_tensor("w_qkv_s", [DIM, 3 * DQ], BF16, kind="ExternalInput").ap()
    b_qkv = nc.dram_tensor("b_qkv_s", [3 * DQ], F32, kind="ExternalInput").ap()
    w_proj = nc.dram_tensor("w_proj_s", [DQ, DIM], BF16, kind="ExternalInput").ap()
    b_proj = nc.dram_tensor("b_proj_h", [DIM], F32, kind="ExternalInput").ap()
    out = nc.dram_tensor("out", [T // 2, DIM], BF16, kind="ExternalOutput").ap()
    partial = nc.dram_tensor("partial", [T, DIM], BF16).ap()
    rs_out = nc.dram_tensor("rs_out", [T // 2, DIM], BF16).ap()

    groups = [[0, 1], [2, 3], [4, 5], [6, 7]]

    def bcast_ap(src_ap, parts):
        # partition-broadcast read of a 1-D DRAM row
        return bass.AP(tensor=src_ap.tensor, offset=src_ap.offset,
                       ap=[[0, parts]] + list(src_ap.ap))

    with tile.TileContext(nc) as tc:
        with (
            tc.tile_pool(name="persist", bufs=1) as pp,
        ):
            k_sb = pp.tile([128, 4, T], BF16)
            v_sb = pp.tile([128, KB_T, HPC, 2 * HD], BF16)
            wq_sb = pp.tile([128, KB_C, DQ], BF16)
            bqkv_sb = pp.tile([128, 12], F32)
            bv_sb = pp.tile([128, DQ], F32)

            nc.vector.memset(v_sb[:, :, :, HD:2 * HD], 1.0)
            warm = pp.tile([128, 512], BF16)
            nc.vector.memset(warm[:], 0.5)
            nc.sync.dma_start(out=bqkv_sb, in_=b_qkv.rearrange("(m p) -> p m", p=128))
            nc.sync.dma_start(out=bv_sb, in_=bcast_ap(b_qkv[2 * DQ:3 * DQ], 128))
            for kb in range(KB_C):
                nc.sync.dma_start(
                    out=wq_sb[:, kb, :],
                    in_=w_qkv[128 * kb:128 * (kb + 1), 0:DQ])

            x_sb = pp.tile([128, KB_C, T], BF16)
            wk_c = pp.tile([128, KB_C, DQ], BF16)
            wv_c = pp.tile([128, KB_C, DQ], BF16)
            for kb in range(KB_C):
                nc.sync.dma_start(
                    out=x_sb[:, kb, 0:1024],
                    in_=x_t[128 * kb:128 * (kb + 1), 0:1024])
                nc.sync.dma_start(
                    out=wk_c[:, kb, :],
                    in_=w_qkv[128 * kb:128 * (kb + 1), DQ:2 * DQ])
            for kb in range(KB_C):
                nc.sync.dma_start(
                    out=x_sb[:, kb, 1024:2048],
                    in_=x_t[128 * kb:128 * (kb + 1), 1024:2048])
                nc.sync.dma_start(
                    out=wv_c[:, kb, :],
                    in_=w_qkv[128 * kb:128 * (kb + 1), 2 * DQ:3 * DQ])

            # HAM warmup happens inside phase B's PSUM budget (pj slots)

            # ---------------- Phase B: Q-proj + attention + proj + RS ------
            with (
                tc.tile_pool(name="zb", bufs=1) as zb,
                tc.tile_pool(name="xsl", bufs=2) as xslp,
                tc.tile_pool(name="qsl", bufs=2) as qslp,
                tc.tile_pool(name="zpool", bufs=3) as zpool,
                tc.tile_pool(name="apool", bufs=5) as apool,
                tc.tile_pool(name="small", bufs=6) as small,
                tc.tile_pool(name="opool", bufs=4) as opool,
                tc.tile_pool(name="psS", bufs=2, space="PSUM") as pss,
                tc.tile_pool(name="psZ", bufs=2, space="PSUM") as psz,
                tc.tile_pool(name="psP", bufs=2, space="PSUM") as psp,
            ):
                wp_sb = zb.tile([128, 4, DIM], BF16)
                bp_sb = zb.tile([128, DIM], F32)
                nc.sync.dma_start(
                    out=wp_sb, in_=w_proj.rearrange("(m p) c -> p m c", p=128))
                nc.sync.dma_start(out=bp_sb, in_=bcast_ap(b_proj[:], 128))

                x_tiles = {}
                q_tiles = {}
                z_tiles = {}

                def emit_x_slice(qb):
                    xs = xslp.tile([128, KB_C, 512], BF16, tag="xs", name=f"xs{qb}")
                    for kb in range(KB_C):
                        nc.sync.dma_start(
                            out=xs[:, kb, :],
                            in_=x_t[128 * kb:128 * (kb + 1),
                                    512 * qb:512 * (qb + 1)])
                    x_tiles[qb] = xs

                def emit_qproj_m(qb, m):
                    if qb == 0:
                        if q0_done[m]:
                            return
                        q0_done[m] = True
                    if m == 0 or qb not in q_tiles:
                        if qb not in q_tiles:
                            q_tiles[qb] = qslp.tile([128, 4, 512], BF16,
                                                    tag="q", name=f"qt{qb}")
                    xs = x_tiles[qb]
                    ps = psp.tile([128, 512], F32, tag="pj")
                    for kb in range(KB_C):
                        nc.tensor.matmul(
                            ps[:],
                            wq_sb[:, kb, 128 * m:128 * (m + 1)],
                            xs[:, kb, :],
                            start=(kb == 0), stop=(kb == KB_C - 1))
                    nc.vector.tensor_scalar_add(
                        out=q_tiles[qb][:, m, :],
                        in0=ps[:],
                        scalar1=bqkv_sb[:, m:m + 1])

                def emit_proj_group(qb, tb4):
                    t0 = 512 * qb + 128 * tb4
                    zt = z_tiles[qb]
                    for cb in range(2):
                        ppj = psp.tile([128, 512], F32, tag="pj")
                        for m in range(4):
                            nc.tensor.matmul(
                                ppj[:],
                                zt[:, m, 128 * tb4:128 * (tb4 + 1)],
                                wp_sb[:, m, 512 * cb:512 * (cb + 1)],
                                start=(m == 0), stop=(m == 3))
                        o = opool.tile([128, 512], BF16, tag="o")
                        nc.vector.tensor_add(
                            o[:], ppj[:], bp_sb[:, 512 * cb:512 * (cb + 1)])
                        nc.sync.dma_start(
                            out=partial[t0:t0 + 128, 512 * cb:512 * (cb + 1)],
                            in_=o[:])

                def emit_rs(qb, tb4=None):
                    if tb4 is None:
                        r0, r1 = 512 * qb, 512 * (qb + 1)
                        o0, o1 = 256 * qb, 256 * (qb + 1)
                    else:
                        r0 = 512 * qb + 128 * tb4
                        r1 = r0 + 128
                        o0 = 256 * qb + 64 * tb4
                        o1 = o0 + 64
                    nc.gpsimd.collective_compute(
                        "ReduceScatter",
                        mybir.AluOpType.add,
                        ins=[partial[r0:r1, :]],
                        outs=[rs_out[o0:o1, :]],
                        replica_groups=groups,
                    )
                    nc.sync.dma_start(out=out[o0:o1, :], in_=rs_out[o0:o1, :])

                v_done = [False] * KB_T
                k_done = [False] * 8
                q0_done = [False] * 4

                def emit_k_half(m, half):
                    tcol = 512 * half
                    psx = psp.tile([128, 512], F32, tag="pj",
                                   name=f"kp{m}_{half}")
                    for kb in range(KB_C):
                        nc.tensor.matmul(
                            psx[:],
                            wk_c[:, kb, 128 * m:128 * (m + 1)],
                            x_sb[:, kb, tcol:tcol + 512],
                            start=(kb == 0), stop=(kb == KB_C - 1))
                    nc.vector.tensor_scalar_add(
                        out=k_sb[:, m, tcol:tcol + 512],
                        in0=psx[:], scalar1=bqkv_sb[:, 4 + m:5 + m])

                def emit_v_unit(tb):
                    if v_done[tb]:
                        return
                    v_done[tb] = True
                    ps = psp.tile([128, DQ], F32, tag="pj", name=f"vps{tb}")
                    for kb in range(KB_C):
                        nc.tensor.matmul(
                            ps[:],
                            x_sb[:, kb, 128 * tb:128 * (tb + 1)],
                            wv_c[:, kb, :],
                            start=(kb == 0), stop=(kb == KB_C - 1))
                    nc.vector.tensor_add(
                        v_sb[:, tb, :, 0:HD],
                        ps[:].rearrange("p (h d) -> p h d", h=HPC),
                        bv_sb[:].rearrange("p (h d) -> p h d", h=HPC))

                def emit_qk(qb, h, ch, s):
                    hp, hh = h // 2, h % 2
                    p0 = 64 * hh
                    qt = q_tiles[qb]
                    for i in range(CH):
                        kc = 128 * (CH * ch + i)
                        nc.tensor.matmul(
                            s[:, i, :],
                            k_sb[p0:p0 + 64, hp, kc:kc + 128],
                            qt[p0:p0 + 64, hp, :],
                            start=True, stop=True)

                def emit_av(h, ch, zz, a):
                    for i in range(CH):
                        kb = CH * ch + i
                        nc.tensor.matmul(
                            zz[:], v_sb[:, kb, h, :],
                            a[:, i, :], start=(kb == 0), stop=(kb == KB_T - 1))

                def emit_zcopy(qb, h):
                    zc = small.tile([128, 512], F32, tag="zc",
                                    name=f"zc{qb}_{h}")
                    nc.vector.tensor_copy(zc[:], z_ps[h][:])
                    zc_tiles[h] = zc

                def emit_norm(qb, h):
                    hp, hh = h // 2, h % 2
                    zc = zc_tiles[h]
                    rinv = small.tile([64, 512], F32, tag="rinv",
                                      name=f"ri{qb}_{h}")
                    nc.vector.reciprocal(rinv[:], zc[64:128, :])
                    nc.gpsimd.tensor_mul(
                        z_tiles[qb][64 * hh:64 * hh + 64, hp, :],
                        zc[0:64, :], rinv[:])

                def emit_service(qb, h):
                    # spread bookkeeping work across the head stream
                    if h == 0 and qb < 3:
                        emit_x_slice(qb + 1)
                    if h % 2 == 0 and qb < 3:
                        emit_qproj_m(qb + 1, h // 2)
                    if h % 2 == 1 and qb > 0:
                        emit_proj_group(qb - 1, h // 2)
                    if h == 7 and qb > 0:
                        emit_rs(qb - 1)

                # prologue: just-in-time K(0, t 0:256) + Q(0, m=0)
                emit_x_slice(0)
                emit_k_half(0, 0)
                emit_qproj_m(0, 0)
                emit_v_unit(0)
                emit_v_unit(1)

                # qb0 service queue: (need-by task index, thunk); K half
                # (m, half) is needed by head 2m once its chunks reach
                # t-columns 512*half; V(tb) by task (h=0, ch=tb//2)
                svc = []
                for tb in range(2, KB_T):
                    svc.append((tb // 2, lambda tb=tb: emit_v_unit(tb)))
                for m in range(4):
                    for half in range(4):
                        if m == 0 and half == 0:
                            continue
                        svc.append((16 * m + 2 * half,
                                    lambda m=m, half=half: emit_k_half(m, half)))
                for m in range(1, 4):
                    svc.append((16 * m, lambda m=m: emit_qproj_m(0, m)))
                svc.sort(key=lambda x: x[0])
                svc_i = [0]

                def emit_service_q0(idx):
                    budget = 2 if idx < 8 else 1
                    while svc_i[0] < len(svc):
                        need, thunk = svc[svc_i[0]]
                        if need <= idx + 1 or (budget > 0 and need <= idx + 10):
                            thunk()
                            svc_i[0] += 1
                            budget -= 1
                        else:
                            break

                NCHUNK = KB_T // CH
                for qb in range(4):
                    z_tiles[qb] = zpool.tile([128, 4, 512], BF16, tag="z",
                                             name=f"zt{qb}")
                    z_ps = {}
                    zc_tiles = {}
                    tasks = [(h, ch) for h in range(8) for ch in range(NCHUNK)]
                    s_tiles = {}
                    s_tiles[0] = pss.tile([128, CH, 512], F32, tag="s",
                                          name=f"s{qb}_0")
                    emit_qk(qb, 0, 0, s_tiles[0])
                    for idx, (h, ch) in enumerate(tasks):
                        if qb == 0:
                            emit_service_q0(idx)
                        if idx + 1 < len(tasks):
                            nh, nch = tasks[idx + 1]
                            s_tiles[idx + 1] = pss.tile(
                                [128, CH, 512], F32, tag="s",
                                name=f"s{qb}_{idx + 1}")
                            emit_qk(qb, nh, nch, s_tiles[idx + 1])
                        a = apool.tile([128, CH, 512], BF16, tag="a",
                                       name=f"a{qb}_{idx}")
                        nc.scalar.activation(out=a[:], in_=s_tiles[idx][:],
                                             func=AF.Exp, scale=SCALE)
                        del s_tiles[idx]
                        if ch == 0:
                            z_ps[h] = psz.tile([128, 512], F32, tag="z",
                                               name=f"zp{qb}_{h}")
                        emit_av(h, ch, z_ps[h], a)
                        if ch == NCHUNK - 1:
                            emit_zcopy(qb, h)
                            if h >= 1:
                                emit_norm(qb, h - 1)
                            emit_service(qb, h)
                    emit_norm(qb, 7)
                for tb4 in range(4):
                    emit_proj_group(3, tb4)
                emit_rs(3)

    nc.compile()
    return nc


def _get_nc():
    if "nc" not in _CACHE:
        _CACHE["nc"] = _build()
    return _CACHE["nc"]


def kernel(x, w_qkv, b_qkv, w_proj, b_proj):
    from concourse.bass_utils import run_bass_kernel_spmd

    x = np.asarray(x, dtype=np.float32)
    w_qkv = np.asarray(w_qkv, dtype=np.float32)
    b_qkv = np.asarray(b_qkv, dtype=np.float32)
    w_proj = np.asarray(w_proj, dtype=np.float32)
    b_proj = np.asarray(b_proj, dtype=np.float32)

    nc = _get_nc()

    in_maps = []
    for c in range(N_CORES):
        b = c // 2
        hg = c % 2
        cols = slice(DQ * hg, DQ * (hg + 1))
        w_s = np.ascontiguousarray(np.concatenate(
            [w_qkv[:, 0:DIM][:, cols],
             w_qkv[:, DIM:2 * DIM][:, cols],
             w_qkv[:, 2 * DIM:3 * DIM][:, cols]], axis=1))
        b_s = np.ascontiguousarray(np.concatenate(
            [b_qkv[0:DIM][cols], b_qkv[DIM:2 * DIM][cols],
             b_qkv[2 * DIM:3 * DIM][cols]]))
        in_maps.append({
            "x_t": np.ascontiguousarray(x[b].T).astype(ml_dtypes.bfloat16),
            "w_qkv_s": w_s.astype(ml_dtypes.bfloat16),
            "b_qkv_s": b_s,
            "w_proj_s": np.ascontiguousarray(
                w_proj[DQ * hg:DQ * (hg + 1), :]).astype(ml_dtypes.bfloat16),
            "b_proj_h": (b_proj * 0.5).astype(np.float32),
        })

    res = run_bass_kernel_spmd(nc, in_maps, core_ids=list(range(N_CORES)))

    full = np.empty((B, T, DIM), dtype=np.float32)
    for c in range(N_CORES):
        b = c // 2
        p = c % 2
        o = np.asarray(res.results[c]["out"]).astype(np.float32)
        for qb in range(4):
            full[b, 512 * qb + 256 * p:512 * qb + 256 * (p + 1), :] = \
                o[256 * qb:256 * (qb + 1), :]
    return full


# revision 28
# speedup vs baseline: 1.0142x; 1.0142x over previous
"""Multi-head attention block on 8 TRN2 NeuronCores.

Sharding: core c -> (batch b = c//2, head-group hg = c%2).
Each core computes QKV projections for its 8 heads over its batch
(fp32r matmuls), attention (fp32r QK^T, exp on ACT, bf16 A@V with a
col-packed ones-matmul producing replicated row-sums), and a bf16
output projection of its head-group's channels. Pairs of cores
(same batch) combine partial projections with per-q-block
ReduceScatter collectives; the host concatenates the 8 per-core
output shards into the full [4, 2048, 1024] result.

Schedule notes: K/V projections form a dense PE preamble; the Q
projection is computed per-q-block inside the attention loop (keeps
the PE near-saturated so the HAM clock gate stays at full rate).
The exp pipeline is staggered head-by-head through a 2-deep PSUM
pool; the previous q-block's output projection and ReduceScatter are
interleaved into the current q-block's head loop through a dedicated
PSUM pool so they never contend with the z accumulators.
"""

import sys

if "/opt/trn_rl_repo" not in sys.path:
    sys.path.insert(0, "/opt/trn_rl_repo")

import numpy as np
import ml_dtypes

N_CORES = 8
B, T, DIM = 4, 2048, 1024
H_TOT, HD = 16, 64
HPC = H_TOT // 2          # heads per core (2 head-groups)
DQ = HPC * HD             # 512: per-core q/k/v width
SCALE = HD ** -0.5
CH = 2                    # k-blocks per exp chunk
KB_T = T // 128           # 16 k-blocks over sequence
KB_C = DIM // 128         # 8 k-blocks over channel dim

_CACHE = {}


def _build():
    import concourse.bass as bass
    import concourse.tile as tile
    from concourse import bacc, mybir

    F32 = mybir.dt.float32
    F32R = mybir.dt.float32r
    BF16 = mybir.dt.bfloat16
    AF = mybir.ActivationFunctionType

    nc = bacc.Bacc("TRN2", target_bir_lowering=False, debug=False,
                   num_devices=N_CORES)

    x_t = nc.dram_tensor("x_t", [DIM, T], BF16, kind="ExternalInput").ap()
    w_qkv = nc.dram_tensor("w_qkv_s", [DIM, 3 * DQ], BF16, kind="ExternalInput").ap()
    b_qkv = nc.dram_tensor("b_qkv_s", [3 * DQ], F32, kind="ExternalInput").ap()
    w_proj = nc.dram_tensor("w_proj_s", [DQ, DIM], BF16, kind="ExternalInput").ap()
    b_proj = nc.dram_tensor("b_proj_h", [DIM], F32, kind="ExternalInput").ap()
    out = nc.dram_tensor("out", [T // 2, DIM], BF16, kind="ExternalOutput").ap()
    partial = nc.dram_tensor("partial", [T, DIM], BF16).ap()
    rs_out = nc.dram_tensor("rs_out", [T // 2, DIM], BF16).ap()

    groups = [[0, 1], [2, 3], [4, 5], [6, 7]]

    def bcast_ap(src_ap, parts):
        # partition-broadcast read of a 1-D DRAM row
        return bass.AP(tensor=src_ap.tensor, offset=src_ap.offset,
                       ap=[[0, parts]] + list(src_ap.ap))

    with tile.TileContext(nc) as tc:
        with (
            tc.tile_pool(name="persist", bufs=1) as pp,
        ):
            k_sb = pp.tile([128, 4, T], BF16)
            v_sb = pp.tile([128, KB_T, HPC, 2 * HD], BF16)
            wq_sb = pp.tile([128, KB_C, DQ], BF16)
            bqkv_sb = pp.tile([128, 12], F32)
            bv_sb = pp.tile([128, DQ], F32)

            nc.vector.memset(v_sb[:, :, :, HD:2 * HD], 1.0)
            warm = pp.tile([128, 512], BF16)
            nc.vector.memset(warm[:], 0.5)
            nc.sync.dma_start(out=bqkv_sb, in_=b_qkv.rearrange("(m p) -> p m", p=128))
            nc.sync.dma_start(out=bv_sb, in_=bcast_ap(b_qkv[2 * DQ:3 * DQ], 128))
            for kb in range(KB_C):
                nc.sync.dma_start(
                    out=wq_sb[:, kb, :],
                    in_=w_qkv[128 * kb:128 * (kb + 1), 0:DQ])

            x_sb = pp.tile([128, KB_C, T], BF16)
            wk_c = pp.tile([128, KB_C, DQ], BF16)
            wv_c = pp.tile([128, KB_C, DQ], BF16)
            for kb in range(KB_C):
                nc.sync.dma_start(
                    out=x_sb[:, kb, 0:1024],
                    in_=x_t[128 * kb:128 * (kb + 1), 0:1024])
                nc.sync.dma_start(
                    out=wk_c[:, kb, :],
                    in_=w_qkv[128 * kb:128 * (kb + 1), DQ:2 * DQ])
            for kb in range(KB_C):
                nc.sync.dma_start(
                    out=x_sb[:, kb, 1024:2048],
                    in_=x_t[128 * kb:128 * (kb + 1), 1024:2048])
                nc.sync.dma_start(
                    out=wv_c[:, kb, :],
                    in_=w_qkv[128 * kb:128 * (kb + 1), 2 * DQ:3 * DQ])

            # HAM warmup happens inside phase B's PSUM budget (pj slots)

            # ---------------- Phase B: Q-proj + attention + proj + RS ------
            with (
                tc.tile_pool(name="zb", bufs=1) as zb,
                tc.tile_pool(name="xsl", bufs=2) as xslp,
                tc.tile_pool(name="qsl", bufs=2) as qslp,
                tc.tile_pool(name="zpool", bufs=3) as zpool,
                tc.tile_pool(name="apool", bufs=5) as apool,
                tc.tile_pool(name="small", bufs=6) as small,
                tc.tile_pool(name="opool", bufs=4) as opool,
                tc.tile_pool(name="psS", bufs=2, space="PSUM") as pss,
                tc.tile_pool(name="psZ", bufs=2, space="PSUM") as psz,
                tc.tile_pool(name="psP", bufs=2, space="PSUM") as psp,
            ):
                wp_sb = zb.tile([128, 4, DIM], BF16)
                bp_sb = zb.tile([128, DIM], F32)
                nc.sync.dma_start(
                    out=wp_sb, in_=w_proj.rearrange("(m p) c -> p m c", p=128))
                nc.sync.dma_start(out=bp_sb, in_=bcast_ap(b_proj[:], 128))

                x_tiles = {}
                q_tiles = {}
                z_tiles = {}

                def emit_x_slice(qb):
                    xs = xslp.tile([128, KB_C, 512], BF16, tag="xs", name=f"xs{qb}")
                    for kb in range(KB_C):
                        nc.sync.dma_start(
                            out=xs[:, kb, :],
                            in_=x_t[128 * kb:128 * (kb + 1),
                                    512 * qb:512 * (qb + 1)])
                    x_tiles[qb] = xs

                def emit_qproj_m(qb, m):
                    if qb == 0:
                        if q0_done[m]:
                            return
                        q0_done[m] = True
                    if m == 0 or qb not in q_tiles:
                        if qb not in q_tiles:
                            q_tiles[qb] = qslp.tile([128, 4, 512], BF16,
                                                    tag="q", name=f"qt{qb}")
                    xs = x_tiles[qb]
                    ps = psp.tile([128, 512], F32, tag="pj")
                    for kb in range(KB_C):
                        nc.tensor.matmul(
                            ps[:],
                            wq_sb[:, kb, 128 * m:128 * (m + 1)],
                            xs[:, kb, :],
                            start=(kb == 0), stop=(kb == KB_C - 1))
                    nc.vector.tensor_scalar_add(
                        out=q_tiles[qb][:, m, :],
                        in0=ps[:],
                        scalar1=bqkv_sb[:, m:m + 1])

                def emit_proj_group(qb, tb4):
                    t0 = 512 * qb + 128 * tb4
                    zt = z_tiles[qb]
                    for cb in range(2):
                        ppj = psp.tile([128, 512], F32, tag="pj")
                        for m in range(4):
                            nc.tensor.matmul(
                                ppj[:],
                                zt[:, m, 128 * tb4:128 * (tb4 + 1)],
                                wp_sb[:, m, 512 * cb:512 * (cb + 1)],
                                start=(m == 0), stop=(m == 3))
                        o = opool.tile([128, 512], BF16, tag="o")
                        nc.vector.tensor_add(
                            o[:], ppj[:], bp_sb[:, 512 * cb:512 * (cb + 1)])
                        nc.sync.dma_start(
                            out=partial[t0:t0 + 128, 512 * cb:512 * (cb + 1)],
                            in_=o[:])

                def emit_rs(qb, tb4=None):
                    if tb4 is None:
                        r0, r1 = 512 * qb, 512 * (qb + 1)
                        o0, o1 = 256 * qb, 256 * (qb + 1)
                    else:
                        r0 = 512 * qb + 128 * tb4
                        r1 = r0 + 128
                        o0 = 256 * qb + 64 * tb4
                        o1 = o0 + 64
                    nc.gpsimd.collective_compute(
                        "ReduceScatter",
                        mybir.AluOpType.add,
                        ins=[partial[r0:r1, :]],
                        outs=[rs_out[o0:o1, :]],
                        replica_groups=groups,
                    )
                    nc.sync.dma_start(out=out[o0:o1, :], in_=rs_out[o0:o1, :])

                v_done = [False] * KB_T
                k_done = [False] * 8
                q0_done = [False] * 4

                def emit_k_half(m, half):
                    tcol = 512 * half
                    psx = psp.tile([128, 512], F32, tag="pj",
                                   name=f"kp{m}_{half}")
                    for kb in range(KB_C):
                        nc.tensor.matmul(
                            psx[:],
                            wk_c[:, kb, 128 * m:128 * (m + 1)],
                            x_sb[:, kb, tcol:tcol + 512],
                            start=(kb == 0), stop=(kb == KB_C - 1))
                    nc.vector.tensor_scalar_add(
                        out=k_sb[:, m, tcol:tcol + 512],
                        in0=psx[:], scalar1=bqkv_sb[:, 4 + m:5 + m])

                def emit_v_unit(tb):
                    if v_done[tb]:
                        return
                    v_done[tb] = True
                    ps = psp.tile([128, DQ], F32, tag="pj", name=f"vps{tb}")
                    for kb in range(KB_C):
                        nc.tensor.matmul(
                            ps[:],
                            x_sb[:, kb, 128 * tb:128 * (tb + 1)],
                            wv_c[:, kb, :],
                            start=(kb == 0), stop=(kb == KB_C - 1))
                    nc.vector.tensor_add(
                        v_sb[:, tb, :, 0:HD],
                        ps[:].rearrange("p (h d) -> p h d", h=HPC),
                        bv_sb[:].rearrange("p (h d) -> p h d", h=HPC))

                def emit_qk(qb, h, ch, s):
                    hp, hh = h // 2, h % 2
                    p0 = 64 * hh
                    qt = q_tiles[qb]
                    for i in range(CH):
                        kc = 128 * (CH * ch + i)
                        nc.tensor.matmul(
                            s[:, i, :],
                            k_sb[p0:p0 + 64, hp, kc:kc + 128],
                            qt[p0:p0 + 64, hp, :],
                            start=True, stop=True)

                def emit_av(h, ch, zz, a):
                    for i in range(CH):
                        kb = CH * ch + i
                        nc.tensor.matmul(
                            zz[:], v_sb[:, kb, h, :],
                            a[:, i, :], start=(kb == 0), stop=(kb == KB_T - 1))

                def emit_zcopy(qb, h):
                    zc = small.tile([128, 512], F32, tag="zc",
                                    name=f"zc{qb}_{h}")
                    nc.vector.tensor_copy(zc[:], z_ps[h][:])
                    zc_tiles[h] = zc

                def emit_norm(qb, h):
                    hp, hh = h // 2, h % 2
                    zc = zc_tiles[h]
                    rinv = small.tile([64, 512], F32, tag="rinv",
                                      name=f"ri{qb}_{h}")
                    nc.vector.reciprocal(rinv[:], zc[64:128, :])
                    nc.gpsimd.tensor_mul(
                        z_tiles[qb][64 * hh:64 * hh + 64, hp, :],
                        zc[0:64, :], rinv[:])

                def emit_service(qb, h):
                    # spread bookkeeping work across the head stream
                    if h == 0 and qb < 3:
                        emit_x_slice(qb + 1)
                    if h % 2 == 0 and qb < 3:
                        emit_qproj_m(qb + 1, h // 2)
                    if h % 2 == 1 and qb > 0:
                        emit_proj_group(qb - 1, h // 2)
                    if h == 7 and qb > 0:
                        emit_rs(qb - 1)

                # prologue: warmup + just-in-time K(0, t 0:256) + Q(0, m=0)
                wps = psp.tile([128, 512], F32, name="warmps", tag="pj")
                for i in range(28):
                    nc.tensor.matmul(wps[:], warm[:, 0:128], warm[:],
                                     start=(i == 0), stop=(i == 27))
                emit_x_slice(0)
                emit_k_half(0, 0)
                emit_qproj_m(0, 0)
                emit_v_unit(0)
                emit_v_unit(1)

                # qb0 service queue: (need-by task index, thunk); K half
                # (m, half) is needed by head 2m once its chunks reach
                # t-columns 512*half; V(tb) by task (h=0, ch=tb//2)
                svc = []
                for tb in range(2, KB_T):
                    svc.append((tb // 2, lambda tb=tb: emit_v_unit(tb)))
                for m in range(4):
                    for half in range(4):
                        if m == 0 and half == 0:
                            continue
                        svc.append((16 * m + 2 * half,
                                    lambda m=m, half=half: emit_k_half(m, half)))
                for m in range(1, 4):
                    svc.append((16 * m, lambda m=m: emit_qproj_m(0, m)))
                svc.sort(key=lambda x: x[0])
                svc_i = [0]

                def emit_service_q0(idx):
                    budget = 2 if idx < 8 else 1
                    while svc_i[0] < len(svc):
                        need, thunk = svc[svc_i[0]]
                        if need <= idx + 1 or (budget > 0 and need <= idx + 10):
                            thunk()
                            svc_i[0] += 1
                            budget -= 1
                        else:
                            break

                NCHUNK = KB_T // CH
                for qb in range(4):
                    z_tiles[qb] = zpool.tile([128, 4, 512], BF16, tag="z",
                                             name=f"zt{qb}")
                    z_ps = {}
                    zc_tiles = {}
                    tasks = [(h, ch) for h in range(8) for ch in range(NCHUNK)]
                    s_tiles = {}
                    s_tiles[0] = pss.tile([128, CH, 512], F32, tag="s",
                                          name=f"s{qb}_0")
                    emit_qk(qb, 0, 0, s_tiles[0])
                    for idx, (h, ch) in enumerate(tasks):
                        if qb == 0:
                            emit_service_q0(idx)
                        if idx + 1 < len(tasks):
                            nh, nch = tasks[idx + 1]
                            s_tiles[idx + 1] = pss.tile(
                                [128, CH, 512], F32, tag="s",
                                name=f"s{qb}_{idx + 1}")
                            emit_qk(qb, nh, nch, s_tiles[idx + 1])
                        a = apool.tile([128, CH, 512], BF16, tag="a",
                                       name=f"a{qb}_{idx}")
                        nc.scalar.activation(out=a[:], in_=s_tiles[idx][:],
                                             func=AF.Exp, scale=SCALE)
                        del s_tiles[idx]
                        if ch == 0:
                            z_ps[h] = psz.tile([128, 512], F32, tag="z",
                                               name=f"zp{qb}_{h}")
                        emit_av(h, ch, z_ps[h], a)
                        if ch == NCHUNK - 1:
                            emit_zcopy(qb, h)
                            if h >= 1:
                                emit_norm(qb, h - 1)
                            emit_service(qb, h)
                    emit_norm(qb, 7)
                for tb4 in range(4):
                    emit_proj_group(3, tb4)
                emit_rs(3)

    nc.compile()
    return nc


def _get_nc():
    if "nc" not in _CACHE:
        _CACHE["nc"] = _build()
    return _CACHE["nc"]


def kernel(x, w_qkv, b_qkv, w_proj, b_proj):
    from concourse.bass_utils import run_bass_kernel_spmd

    x = np.asarray(x, dtype=np.float32)
    w_qkv = np.asarray(w_qkv, dtype=np.float32)
    b_qkv = np.asarray(b_qkv, dtype=np.float32)
    w_proj = np.asarray(w_proj, dtype=np.float32)
    b_proj = np.asarray(b_proj, dtype=np.float32)

    nc = _get_nc()

    in_maps = []
    for c in range(N_CORES):
        b = c // 2
        hg = c % 2
        cols = slice(DQ * hg, DQ * (hg + 1))
        w_s = np.ascontiguousarray(np.concatenate(
            [w_qkv[:, 0:DIM][:, cols],
             w_qkv[:, DIM:2 * DIM][:, cols],
             w_qkv[:, 2 * DIM:3 * DIM][:, cols]], axis=1))
        b_s = np.ascontiguousarray(np.concatenate(
            [b_qkv[0:DIM][cols], b_qkv[DIM:2 * DIM][cols],
             b_qkv[2 * DIM:3 * DIM][cols]]))
        in_maps.append({
            "x_t": np.ascontiguousarray(x[b].T).astype(ml_dtypes.bfloat16),
            "w_qkv_s": w_s.astype(ml_dtypes.bfloat16),
            "b_qkv_s": b_s,
            "w_proj_s": np.ascontiguousarray(
                w_proj[DQ * hg:DQ * (hg + 1), :]).astype(ml_dtypes.bfloat16),
            "b_proj_h": (b_proj * 0.5).astype(np.float32),
        })

    res = run_bass_kernel_spmd(nc, in_maps, core_ids=list(range(N_CORES)))

    full = np.empty((B, T, DIM), dtype=np.float32)
    for c in range(N_CORES):
        b = c // 2
        p = c % 2
        o = np.asarray(res.results[c]["out"]).astype(np.float32)
        for qb in range(4):
            full[b, 512 * qb + 256 * p:512 * qb + 256 * (p + 1), :] = \
                o[256 * qb:256 * (qb + 1), :]
    return full


# revision 29
# speedup vs baseline: 1.0831x; 1.0679x over previous
"""Multi-head attention block on 8 TRN2 NeuronCores.

Sharding: core c -> (batch b = c//2, head-group hg = c%2).
Each core computes QKV projections for its 8 heads over its batch
(fp32r matmuls), attention (fp32r QK^T, exp on ACT, bf16 A@V with a
col-packed ones-matmul producing replicated row-sums), and a bf16
output projection of its head-group's channels. Pairs of cores
(same batch) combine partial projections with per-q-block
ReduceScatter collectives; the host concatenates the 8 per-core
output shards into the full [4, 2048, 1024] result.

Schedule notes: K/V projections form a dense PE preamble; the Q
projection is computed per-q-block inside the attention loop (keeps
the PE near-saturated so the HAM clock gate stays at full rate).
The exp pipeline is staggered head-by-head through a 2-deep PSUM
pool; the previous q-block's output projection and ReduceScatter are
interleaved into the current q-block's head loop through a dedicated
PSUM pool so they never contend with the z accumulators.
"""

import sys

if "/opt/trn_rl_repo" not in sys.path:
    sys.path.insert(0, "/opt/trn_rl_repo")

import numpy as np
import ml_dtypes

N_CORES = 8
B, T, DIM = 4, 2048, 1024
H_TOT, HD = 16, 64
HPC = H_TOT // 2          # heads per core (2 head-groups)
DQ = HPC * HD             # 512: per-core q/k/v width
SCALE = HD ** -0.5
CH = 2                    # k-blocks per exp chunk
KB_T = T // 128           # 16 k-blocks over sequence
KB_C = DIM // 128         # 8 k-blocks over channel dim

_CACHE = {}


def _build():
    import concourse.bass as bass
    import concourse.tile as tile
    from concourse import bacc, mybir

    F32 = mybir.dt.float32
    F32R = mybir.dt.float32r
    BF16 = mybir.dt.bfloat16
    AF = mybir.ActivationFunctionType

    nc = bacc.Bacc("TRN2", target_bir_lowering=False, debug=False,
                   num_devices=N_CORES)

    x_t = nc.dram_tensor("x_t", [DIM, T], BF16, kind="ExternalInput").ap()
    w_qkv = nc.dram_tensor("w_qkv_s", [DIM, 3 * DQ], BF16, kind="ExternalInput").ap()
    b_qkv = nc.dram_tensor("b_qkv_s", [3 * DQ], F32, kind="ExternalInput").ap()
    w_proj = nc.dram_tensor("w_proj_s", [DQ, DIM], BF16, kind="ExternalInput").ap()
    b_proj = nc.dram_tensor("b_proj_h", [DIM], F32, kind="ExternalInput").ap()
    out = nc.dram_tensor("out", [T // 2, DIM], BF16, kind="ExternalOutput").ap()
    partial = nc.dram_tensor("partial", [T, DIM], BF16).ap()
    rs_out = nc.dram_tensor("rs_out", [T // 2, DIM], BF16).ap()

    groups = [[0, 1], [2, 3], [4, 5], [6, 7]]

    def bcast_ap(src_ap, parts):
        # partition-broadcast read of a 1-D DRAM row
        return bass.AP(tensor=src_ap.tensor, offset=src_ap.offset,
                       ap=[[0, parts]] + list(src_ap.ap))

    with tile.TileContext(nc) as tc:
        with (
            tc.tile_pool(name="persist", bufs=1) as pp,
        ):
            k_sb = pp.tile([128, 4, T], BF16)
            v_sb = pp.tile([128, KB_T, HPC, 2 * HD], BF16)
            wq_sb = pp.tile([128, KB_C, DQ], BF16)
            bqkv_sb = pp.tile([128, 12], F32)
            bv_sb = pp.tile([128, DQ], F32)

            nc.vector.memset(v_sb[:, :, :, HD:2 * HD], 1.0)
            warm = pp.tile([128, 512], BF16)
            nc.vector.memset(warm[:], 0.5)
            nc.sync.dma_start(out=bqkv_sb, in_=b_qkv.rearrange("(m p) -> p m", p=128))
            nc.sync.dma_start(out=bv_sb, in_=bcast_ap(b_qkv[2 * DQ:3 * DQ], 128))
            for kb in range(KB_C):
                nc.sync.dma_start(
                    out=wq_sb[:, kb, :],
                    in_=w_qkv[128 * kb:128 * (kb + 1), 0:DQ])

            x_sb = pp.tile([128, KB_C, T], BF16)
            wk_c = pp.tile([128, KB_C, DQ], BF16)
            wv_c = pp.tile([128, KB_C, DQ], BF16)
            for kb in range(KB_C):
                nc.sync.dma_start(
                    out=x_sb[:, kb, 0:1024],
                    in_=x_t[128 * kb:128 * (kb + 1), 0:1024])
                nc.sync.dma_start(
                    out=wk_c[:, kb, :],
                    in_=w_qkv[128 * kb:128 * (kb + 1), DQ:2 * DQ])
            for kb in range(KB_C):
                nc.sync.dma_start(
                    out=x_sb[:, kb, 1024:2048],
                    in_=x_t[128 * kb:128 * (kb + 1), 1024:2048])
                nc.sync.dma_start(
                    out=wv_c[:, kb, :],
                    in_=w_qkv[128 * kb:128 * (kb + 1), 2 * DQ:3 * DQ])

            # HAM warmup happens inside phase B's PSUM budget (pj slots)

            # ---------------- Phase B: Q-proj + attention + proj + RS ------
            with (
                tc.tile_pool(name="zb", bufs=1) as zb,
                tc.tile_pool(name="xsl", bufs=2) as xslp,
                tc.tile_pool(name="qsl", bufs=2) as qslp,
                tc.tile_pool(name="zpool", bufs=3) as zpool,
                tc.tile_pool(name="apool", bufs=5) as apool,
                tc.tile_pool(name="small", bufs=6) as small,
                tc.tile_pool(name="opool", bufs=4) as opool,
                tc.tile_pool(name="psS", bufs=2, space="PSUM") as pss,
                tc.tile_pool(name="psZ", bufs=2, space="PSUM") as psz,
                tc.tile_pool(name="psP", bufs=2, space="PSUM") as psp,
            ):
                wp_sb = zb.tile([128, 4, DIM], BF16)
                bp_sb = zb.tile([128, DIM], F32)
                nc.sync.dma_start(
                    out=wp_sb, in_=w_proj.rearrange("(m p) c -> p m c", p=128))
                nc.sync.dma_start(out=bp_sb, in_=bcast_ap(b_proj[:], 128))

                x_tiles = {}
                q_tiles = {}
                z_tiles = {}

                def emit_x_slice(qb):
                    xs = xslp.tile([128, KB_C, 512], BF16, tag="xs", name=f"xs{qb}")
                    for kb in range(KB_C):
                        nc.sync.dma_start(
                            out=xs[:, kb, :],
                            in_=x_t[128 * kb:128 * (kb + 1),
                                    512 * qb:512 * (qb + 1)])
                    x_tiles[qb] = xs

                def emit_qproj_m(qb, m):
                    if qb == 0:
                        if q0_done[m]:
                            return
                        q0_done[m] = True
                    if m == 0 or qb not in q_tiles:
                        if qb not in q_tiles:
                            q_tiles[qb] = qslp.tile([128, 4, 512], BF16,
                                                    tag="q", name=f"qt{qb}")
                    xs = x_tiles[qb]
                    ps = psp.tile([128, 512], F32, tag="pj")
                    for kb in range(KB_C):
                        nc.tensor.matmul(
                            ps[:],
                            wq_sb[:, kb, 128 * m:128 * (m + 1)],
                            xs[:, kb, :],
                            start=(kb == 0), stop=(kb == KB_C - 1))
                    nc.vector.tensor_scalar_add(
                        out=q_tiles[qb][:, m, :],
                        in0=ps[:],
                        scalar1=bqkv_sb[:, m:m + 1])

                def emit_proj_group(qb, tb4):
                    t0 = 512 * qb + 128 * tb4
                    zt = z_tiles[qb]
                    for cb in range(2):
                        ppj = psp.tile([128, 512], F32, tag="pj")
                        for m in range(4):
                            nc.tensor.matmul(
                                ppj[:],
                                zt[:, m, 128 * tb4:128 * (tb4 + 1)],
                                wp_sb[:, m, 512 * cb:512 * (cb + 1)],
                                start=(m == 0), stop=(m == 3))
                        o = opool.tile([128, 512], BF16, tag="o")
                        nc.vector.tensor_add(
                            o[:], ppj[:], bp_sb[:, 512 * cb:512 * (cb + 1)])
                        nc.sync.dma_start(
                            out=partial[t0:t0 + 128, 512 * cb:512 * (cb + 1)],
                            in_=o[:])

                def emit_rs(qb, tb4=None):
                    if tb4 is None:
                        r0, r1 = 512 * qb, 512 * (qb + 1)
                        o0, o1 = 256 * qb, 256 * (qb + 1)
                    else:
                        r0 = 512 * qb + 128 * tb4
                        r1 = r0 + 128
                        o0 = 256 * qb + 64 * tb4
                        o1 = o0 + 64
                    nc.gpsimd.collective_compute(
                        "ReduceScatter",
                        mybir.AluOpType.add,
                        ins=[partial[r0:r1, :]],
                        outs=[rs_out[o0:o1, :]],
                        replica_groups=groups,
                    )
                    nc.sync.dma_start(out=out[o0:o1, :], in_=rs_out[o0:o1, :])

                v_done = [False] * KB_T
                k_done = [False] * 8
                q0_done = [False] * 4

                def emit_k_half(m, half):
                    tcol = 512 * half
                    psx = psp.tile([128, 512], F32, tag="pj",
                                   name=f"kp{m}_{half}")
                    for kb in range(KB_C):
                        nc.tensor.matmul(
                            psx[:],
                            wk_c[:, kb, 128 * m:128 * (m + 1)],
                            x_sb[:, kb, tcol:tcol + 512],
                            start=(kb == 0), stop=(kb == KB_C - 1))
                    nc.vector.tensor_scalar_add(
                        out=k_sb[:, m, tcol:tcol + 512],
                        in0=psx[:], scalar1=bqkv_sb[:, 4 + m:5 + m])

                def emit_v_unit(tb):
                    if v_done[tb]:
                        return
                    v_done[tb] = True
                    ps = psp.tile([128, DQ], F32, tag="pj", name=f"vps{tb}")
                    for kb in range(KB_C):
                        nc.tensor.matmul(
                            ps[:],
                            x_sb[:, kb, 128 * tb:128 * (tb + 1)],
                            wv_c[:, kb, :],
                            start=(kb == 0), stop=(kb == KB_C - 1))
                    nc.vector.tensor_add(
                        v_sb[:, tb, :, 0:HD],
                        ps[:].rearrange("p (h d) -> p h d", h=HPC),
                        bv_sb[:].rearrange("p (h d) -> p h d", h=HPC))

                def emit_qk_pair(qb, hp, kb, s):
                    qt = q_tiles[qb]
                    kc = 128 * kb
                    nc.tensor.matmul(
                        s[:, 0, :],
                        k_sb[0:64, hp, kc:kc + 128],
                        qt[0:64, hp, :],
                        start=True, stop=True)
                    nc.tensor.matmul(
                        s[:, 1, :],
                        k_sb[64:128, hp, kc:kc + 128],
                        qt[64:128, hp, :],
                        start=True, stop=True)

                def emit_av_pair(hp, kb, z0, z1, a):
                    st = (kb == 0)
                    sp = (kb == KB_T - 1)
                    nc.tensor.matmul(
                        z0[:], v_sb[:, kb, 2 * hp, :],
                        a[:, 0, :], start=st, stop=sp)
                    nc.tensor.matmul(
                        z1[:], v_sb[:, kb, 2 * hp + 1, :],
                        a[:, 1, :], start=st, stop=sp)

                def emit_zcopy(qb, h):
                    zc = small.tile([128, 512], F32, tag="zc",
                                    name=f"zc{qb}_{h}")
                    nc.vector.tensor_copy(zc[:], z_ps[h][:])
                    zc_tiles[h] = zc

                def emit_norm(qb, h):
                    hp, hh = h // 2, h % 2
                    zc = zc_tiles[h]
                    rinv = small.tile([64, 512], F32, tag="rinv",
                                      name=f"ri{qb}_{h}")
                    nc.vector.reciprocal(rinv[:], zc[64:128, :])
                    nc.gpsimd.tensor_mul(
                        z_tiles[qb][64 * hh:64 * hh + 64, hp, :],
                        zc[0:64, :], rinv[:])

                def emit_service(qb, h):
                    # spread bookkeeping work across the head stream
                    if h == 0 and qb < 3:
                        emit_x_slice(qb + 1)
                    if h % 2 == 0 and qb < 3:
                        emit_qproj_m(qb + 1, h // 2)
                    if h % 2 == 1 and qb > 0:
                        emit_proj_group(qb - 1, h // 2)
                    if h == 7 and qb > 0:
                        emit_rs(qb - 1)

                # prologue: warmup + just-in-time K(0, t 0:256) + Q(0, m=0)
                wps = psp.tile([128, 512], F32, name="warmps", tag="pj")
                for i in range(28):
                    nc.tensor.matmul(wps[:], warm[:, 0:128], warm[:],
                                     start=(i == 0), stop=(i == 27))
                emit_x_slice(0)
                emit_k_half(0, 0)
                emit_qproj_m(0, 0)
                emit_v_unit(0)
                emit_v_unit(1)

                # qb0 service queue: (need-by task index, thunk); K half
                # (m, half) is needed by head 2m once its chunks reach
                # t-columns 512*half; V(tb) by task (h=0, ch=tb//2)
                svc = []
                for tb in range(2, KB_T):
                    svc.append((tb, lambda tb=tb: emit_v_unit(tb)))
                for m in range(4):
                    for half in range(4):
                        if m == 0 and half == 0:
                            continue
                        svc.append((16 * m + 4 * half - 1,
                                    lambda m=m, half=half: emit_k_half(m, half)))
                for m in range(1, 4):
                    svc.append((16 * m - 1, lambda m=m: emit_qproj_m(0, m)))
                svc.sort(key=lambda x: x[0])
                svc_i = [0]

                def emit_service_q0(idx):
                    budget = 2 if idx < 8 else 1
                    while svc_i[0] < len(svc):
                        need, thunk = svc[svc_i[0]]
                        if need <= idx + 1 or (budget > 0 and need <= idx + 10):
                            thunk()
                            svc_i[0] += 1
                            budget -= 1
                        else:
                            break

                for qb in range(4):
                    z_tiles[qb] = zpool.tile([128, 4, 512], BF16, tag="z",
                                             name=f"zt{qb}")
                    z_ps = {}
                    zc_tiles = {}
                    tasks = [(hp, kb) for hp in range(4) for kb in range(KB_T)]
                    s_tiles = {}
                    s_tiles[0] = pss.tile([128, 2, 512], F32, tag="s",
                                          name=f"s{qb}_0")
                    emit_qk_pair(qb, 0, 0, s_tiles[0])
                    for idx, (hp, kb) in enumerate(tasks):
                        if qb == 0:
                            emit_service_q0(idx)
                        if idx + 1 < len(tasks):
                            nhp, nkb = tasks[idx + 1]
                            s_tiles[idx + 1] = pss.tile(
                                [128, 2, 512], F32, tag="s",
                                name=f"s{qb}_{idx + 1}")
                            emit_qk_pair(qb, nhp, nkb, s_tiles[idx + 1])
                        a = apool.tile([128, 2, 512], BF16, tag="a",
                                       name=f"a{qb}_{idx}")
                        nc.scalar.activation(out=a[:], in_=s_tiles[idx][:],
                                             func=AF.Exp, scale=SCALE)
                        del s_tiles[idx]
                        if kb == 0:
                            z_ps[2 * hp] = psz.tile([128, 512], F32, tag="z",
                                                    name=f"zp{qb}_{2 * hp}")
                            z_ps[2 * hp + 1] = psz.tile(
                                [128, 512], F32, tag="z",
                                name=f"zp{qb}_{2 * hp + 1}")
                        emit_av_pair(hp, kb, z_ps[2 * hp], z_ps[2 * hp + 1], a)
                        if kb == KB_T // 2 - 1:
                            emit_service(qb, 2 * hp)
                        if kb == KB_T - 1:
                            emit_zcopy(qb, 2 * hp)
                            emit_zcopy(qb, 2 * hp + 1)
                            if hp >= 1:
                                emit_norm(qb, 2 * (hp - 1))
                                emit_norm(qb, 2 * (hp - 1) + 1)
                            emit_service(qb, 2 * hp + 1)
                    emit_norm(qb, 6)
                    emit_norm(qb, 7)
                for tb4 in range(4):
                    emit_proj_group(3, tb4)
                emit_rs(3)

    nc.compile()
    return nc


def _get_nc():
    if "nc" not in _CACHE:
        _CACHE["nc"] = _build()
    return _CACHE["nc"]


def kernel(x, w_qkv, b_qkv, w_proj, b_proj):
    from concourse.bass_utils import run_bass_kernel_spmd

    x = np.asarray(x, dtype=np.float32)
    w_qkv = np.asarray(w_qkv, dtype=np.float32)
    b_qkv = np.asarray(b_qkv, dtype=np.float32)
    w_proj = np.asarray(w_proj, dtype=np.float32)
    b_proj = np.asarray(b_proj, dtype=np.float32)

    nc = _get_nc()

    in_maps = []
    for c in range(N_CORES):
        b = c // 2
        hg = c % 2
        cols = slice(DQ * hg, DQ * (hg + 1))
        w_s = np.ascontiguousarray(np.concatenate(
            [w_qkv[:, 0:DIM][:, cols],
             w_qkv[:, DIM:2 * DIM][:, cols],
             w_qkv[:, 2 * DIM:3 * DIM][:, cols]], axis=1))
        b_s = np.ascontiguousarray(np.concatenate(
            [b_qkv[0:DIM][cols], b_qkv[DIM:2 * DIM][cols],
             b_qkv[2 * DIM:3 * DIM][cols]]))
        in_maps.append({
            "x_t": np.ascontiguousarray(x[b].T).astype(ml_dtypes.bfloat16),
            "w_qkv_s": w_s.astype(ml_dtypes.bfloat16),
            "b_qkv_s": b_s,
            "w_proj_s": np.ascontiguousarray(
                w_proj[DQ * hg:DQ * (hg + 1), :]).astype(ml_dtypes.bfloat16),
            "b_proj_h": (b_proj * 0.5).astype(np.float32),
        })

    res = run_bass_kernel_spmd(nc, in_maps, core_ids=list(range(N_CORES)))

    full = np.empty((B, T, DIM), dtype=np.float32)
    for c in range(N_CORES):
        b = c // 2
        p = c % 2
        o = np.asarray(res.results[c]["out"]).astype(np.float32)
        for qb in range(4):
            full[b, 512 * qb + 256 * p:512 * qb + 256 * (p + 1), :] = \
                o[256 * qb:256 * (qb + 1), :]
    return full
